# revision 25
# baseline (speedup 1.0000x reference)
"""Trainium2 Bass kernel for db4 wavelet high-frequency extraction (v2, fp16).

Math: per (b,c) plane X [512,512]:
    out = 2X + D (E-I)^T,   D = (I-E) X,   E = S_hi @ G_hi (bandwidth 7).

Device computation (per plane, all fp16 except PSUM):
  upload X2 = 2X as fp16, partition-major layout [128, plane, 4*512]
  stage 1: D2^T = X2^T B^T  (B = I-E) via banded matmuls: per 128-col strip,
           4 exact-support matmuls (548 PE cycles vs 1024 for 256-windows).
           PSUM lazy-zero semantics (start=True on the first matmul marks the
           whole bank; later matmuls overwrite pending / accumulate written
           elements) handle the band overlaps with no extra work.
  stage 2: ps = D2 (0.5(E-I))^T via the same banded scheme, then
           DVE tensor_add: out = ps + X2  (the 2X term, no identity matmul).
           Stage-1 evictions on ACT, [128,1024] 2-bank ops; output DMA
           triggered from the idle gpsimd queue (no FIFO head-of-line).
  download fp16, host converts to f32.

I/O is fp16 -> 12.58 MB per core per pass -> ~35 us HBM floor (vs 70 f32).
Per-plane emission is software-pipelined (stage2 of plane p-1 after stage1
of plane p); the timing variant unrolls UNROLL passes per For_i iteration
to amortize the loop's all-engine barrier.

Sharding: 96 (b,c) planes, 12 per core, pure data parallel on 8 cores.
"""
import numpy as np

# ---------------------------------------------------------------- constants
_DEC_LO = np.array([-0.010597401784997278, 0.032883011666982945,
                    0.030841381835986965, -0.18703481171888114,
                    -0.02798376941698385, 0.6308807679295904,
                    0.7148465705525415, 0.23037781330885523], dtype=np.float64)
_F = 8
_SIGNS = np.array([(-1.0) ** (k + 1) for k in range(_F)])
_DEC_HI = _SIGNS * _DEC_LO[::-1]
_REC_LO = _DEC_LO[::-1].copy()
_REC_HI = _DEC_HI[::-1].copy()

N = 512
M = (N + _F - 1) // 2
B_TOT, C_TOT, PLANES_PER_CORE, N_CORES = 32, 3, 12, 8
UNROLL = 24  # passes per For_i iteration in the dynamic (timing) variant

# band-block supports (BT/MT block rc nonzero cols = [SUPP[rc][0], SUPP[rc][1]))
SUPP = [(0, 134), (122, 262), (250, 390), (378, 512)]
STORE = SUPP  # stored col range per block = exact support
SBW = 144  # padded stored band width per block
# issue order: (rc, n_lo, n_hi, start).  PSUM hardware semantics: start=True
# on the FIRST matmul lazily zeroes the whole 2KB bank (per-element
# has_written bits); every later matmul overwrites still-pending elements
# and accumulates onto written ones.  So one start per bank + exact-support
# matmuls handle the band overlaps automatically, and the union of the four
# supports covers all 512 columns.
PLAN = [(0, 0, 134, True), (1, 122, 262, False),
        (2, 250, 390, False), (3, 378, 512, False)]


def _dwt_matrices(n):
    m = (n + _F - 1) // 2
    idx = np.concatenate([np.arange(_F - 2, -1, -1), np.arange(n),
                          np.arange(n - 1, n - _F, -1)])[1:]
    G_lo = np.zeros((m, n))
    G_hi = np.zeros((m, n))
    rev_lo = _DEC_LO[::-1]
    rev_hi = _DEC_HI[::-1]
    for i in range(m):
        for k in range(_F):
            t = 2 * i + k
            G_lo[i, idx[t]] += rev_lo[k]
            G_hi[i, idx[t]] += rev_hi[k]
    return G_lo, G_hi


def _idwt_matrices(n, m):
    up_len = 2 * m - 1
    S_lo = np.zeros((n, m))
    S_hi = np.zeros((n, m))
    for i in range(n):
        t = i + _F - 2
        for j_up in range(max(0, t - _F + 1), min(up_len, t + 1)):
            k = t - j_up
            if j_up % 2 == 0:
                S_lo[i, j_up // 2] += _REC_LO[k]
                S_hi[i, j_up // 2] += _REC_HI[k]
    return S_lo, S_hi


def _build_streams():
    """Returns s1, s2 [4,128,SBW] fp16: B^T and 0.5(E-I)^T band blocks."""
    _, G_hi = _dwt_matrices(N)
    _, S_hi = _idwt_matrices(N, M)
    E = S_hi @ G_hi
    BT = (np.eye(N) - E).T
    MT = 0.5 * (E - np.eye(N)).T
    s1 = np.zeros((4, 128, SBW), dtype=np.float16)
    s2 = np.zeros((4, 128, SBW), dtype=np.float16)
    for rc, (lo, hi) in enumerate(STORE):
        s1[rc, :, :hi - lo] = BT[rc * 128:(rc + 1) * 128, lo:hi]
        s2[rc, :, :hi - lo] = MT[rc * 128:(rc + 1) * 128, lo:hi]
    return s1, s2


_PLAN = PLAN


# ---------------------------------------------------------------- bass build
_NC_CACHE = {}


def _build_nc(reps=1, dynamic=False):
    import contextlib
    import concourse.bacc as bacc
    import concourse.mybir as mybir
    from concourse.tile import TileContext

    F16 = mybir.dt.float16
    F32 = mybir.dt.float32
    P = PLANES_PER_CORE

    nc = bacc.Bacc(None)
    # partition-major: data_d[p, plane, rc*512 + c] = 2*X[plane, rc*128+p, c]
    data_d = nc.declare_dram_parameter("data", [128, P, 4 * N], F16,
                                       isOutput=False)
    s1_d = nc.declare_dram_parameter("s1", [4, 128, SBW], F16, isOutput=False)
    s2_d = nc.declare_dram_parameter("s2", [4, 128, SBW], F16, isOutput=False)
    out_d = nc.declare_dram_parameter("out", [128, P, 4 * N], F16,
                                      isOutput=True)

    with TileContext(nc) as tc:
        with (
            tc.tile_pool(name="const", bufs=1) as cpool,
            tc.tile_pool(name="xin", bufs=10) as xin,
            tc.tile_pool(name="mid", bufs=4) as mid,
            tc.tile_pool(name="oout", bufs=4) as oout,
            tc.tile_pool(name="ps", bufs=2, space="PSUM") as ps,
        ):
            s1_sb = cpool.tile([128, 4, SBW], F16)
            s2_sb = cpool.tile([128, 4, SBW], F16)
            nc.sync.dma_start(out=s1_sb[:], in_=s1_d[:].rearrange("rc p w -> p rc w"))
            nc.sync.dma_start(out=s2_sb[:], in_=s2_d[:].rearrange("rc p w -> p rc w"))

            def stage1(plane):
                """Load plane, run D2^T = X2^T B^T, evict to SBUF (ACT)."""
                x_sb = xin.tile([128, 4 * N], F16, tag="x")
                # one whole-plane load (4KB contiguous per partition);
                # alternate the two HWDGE rings (SP / ACT) so descriptor
                # generation and engine packet interleave parallelize
                q = nc.sync if plane % 2 == 0 else nc.scalar
                q.dma_start(out=x_sb[:], in_=data_d[:, plane, :])
                d2t_sb = mid.tile([128, 4 * N], F16, tag="d2t")
                # 2-bank PSUM tiles; one big ACT eviction per strip-pair
                # (DVE is reserved for the stage-2 adds)
                for wq in range(2):
                    ps_t = ps.tile([128, 2 * N], F32, tag="ps_t")
                    for sub in range(2):
                        wc = 2 * wq + sub
                        for i, (rc, n0, n1, st) in enumerate(_PLAN):
                            nc.tensor.matmul(
                                ps_t[:, sub * N + n0:sub * N + n1],
                                x_sb[:, rc * N + wc * 128:rc * N + (wc + 1) * 128],
                                s1_sb[:, rc, n0 - STORE[rc][0]:n1 - STORE[rc][0]],
                                start=st, stop=(i == len(_PLAN) - 1))
                    nc.scalar.copy(
                        d2t_sb[:, wq * 2 * N:(wq + 1) * 2 * N], ps_t[:])
                return x_sb, d2t_sb

            def stage2(plane, x_sb, d2t_sb):
                """out = X2 + D2 (0.5(E-I))^T, DVE add."""
                o_sb = oout.tile([128, 4 * N], F16, tag="o")
                for iq in range(2):
                    ps_o = ps.tile([128, 2 * N], F32, tag="ps_o")
                    for sub in range(2):
                        ic = 2 * iq + sub
                        for i, (kc, n0, n1, st) in enumerate(_PLAN):
                            nc.tensor.matmul(
                                ps_o[:, sub * N + n0:sub * N + n1],
                                d2t_sb[:, kc * N + ic * 128:kc * N + (ic + 1) * 128],
                                s2_sb[:, kc, n0 - STORE[kc][0]:n1 - STORE[kc][0]],
                                start=st, stop=(i == len(_PLAN) - 1))
                    nc.vector.tensor_add(
                        o_sb[:, iq * 2 * N:(iq + 1) * 2 * N], ps_o[:],
                        x_sb[:, iq * 2 * N:(iq + 1) * 2 * N])
                # output store triggered from the (otherwise idle) gpsimd
                # queue so its wait-for-adds never head-of-line blocks the
                # ACT copies or SP loads
                nc.gpsimd.dma_start(out=out_d[:, plane, :], in_=o_sb[:])

            def one_pass():
                # software pipeline: emit stage2(p-1) after stage1(p) so the
                # strict-FIFO PE queue always has ready work (stage2's matmuls
                # wait on copies that complete during the next stage1)
                prev = None
                for plane in range(P):
                    cur = (plane, *stage1(plane))
                    if prev is not None:
                        stage2(*prev)
                    prev = cur
                stage2(*prev)

            if dynamic:
                # For_i has an all-engine barrier per iteration; unroll
                # UNROLL passes per iteration so the barrier + pipeline
                # fill/drain amortizes and passes overlap via pool rotation.
                # reps = UNROLL * n_iters must hold.
                assert reps % UNROLL == 0, (reps, UNROLL)
                with tc.For_i(0, reps // UNROLL, 1):
                    for _ in range(UNROLL):
                        one_pass()
            else:
                for _ in range(reps):
                    one_pass()

    nc.finalize()
    return nc


def _get_nc(reps=1, dynamic=False):
    key = (reps, dynamic)
    if key not in _NC_CACHE:
        _NC_CACHE[key] = _build_nc(reps, dynamic)
    return _NC_CACHE[key]


_STREAMS = None


def _get_streams():
    global _STREAMS
    if _STREAMS is None:
        _STREAMS = _build_streams()
    return _STREAMS


_RUNNERS = {}


def _make_runner(reps=1, dynamic=False):
    """Build a persistent jitted SPMD callable for the kernel program."""
    import jax
    import numpy as _np
    from jax.sharding import Mesh, PartitionSpec
    from jax.experimental.shard_map import shard_map
    import concourse.mybir as mybir
    from concourse import bass2jax

    bass2jax.install_neuronx_cc_hook()
    nc = _get_nc(reps, dynamic)

    partition_name = (nc.partition_id_tensor.name
                      if nc.partition_id_tensor else None)
    in_names, out_names, out_avals, zero_outs = [], [], [], []
    for alloc in nc.m.functions[0].allocations:
        if not isinstance(alloc, mybir.MemoryLocationSet):
            continue
        name = alloc.memorylocations[0].name
        if alloc.kind == "ExternalInput":
            if name != partition_name:
                in_names.append(name)
        elif alloc.kind == "ExternalOutput":
            out_names.append(name)
            shape = tuple(alloc.tensor_shape)
            dtype = mybir.dt.np(alloc.dtype)
            out_avals.append(jax.core.ShapedArray(shape, dtype))
            zero_outs.append(_np.zeros(shape, dtype))
    n_params = len(in_names)
    n_outs = len(out_avals)
    all_in_names = in_names + out_names
    if partition_name is not None:
        all_in_names.append(partition_name)
    donate = tuple(range(n_params, n_params + n_outs))

    def _body(*args):
        operands = list(args)
        if partition_name is not None:
            operands.append(bass2jax.partition_id_tensor())
        outs = bass2jax._bass_exec_p.bind(
            *operands,
            out_avals=tuple(out_avals),
            in_names=tuple(all_in_names),
            out_names=tuple(out_names),
            lowering_input_output_aliases=(),
            sim_require_finite=True,
            sim_require_nnan=True,
            nc=nc,
        )
        return tuple(outs)

    devices = jax.devices()[:N_CORES]
    mesh = Mesh(np.asarray(devices), ("core",))
    in_specs = (PartitionSpec("core"),) * (n_params + n_outs)
    out_specs = (PartitionSpec("core"),) * n_outs
    sharded = jax.jit(
        shard_map(_body, mesh=mesh, in_specs=in_specs, out_specs=out_specs,
                  check_rep=False),
        donate_argnums=donate, keep_unused=True)

    def _concat_in(per_core_inputs):
        return [
            _np.concatenate([_np.asarray(per_core_inputs[c][nm])
                             for c in range(N_CORES)], axis=0)
            for nm in in_names
        ]

    def run(per_core_inputs):
        """per_core_inputs: list over cores of dict name->np array."""
        concat_zeros = [
            _np.zeros((N_CORES * z.shape[0], *z.shape[1:]), z.dtype)
            for z in zero_outs
        ]
        out_arrs = sharded(*_concat_in(per_core_inputs), *concat_zeros)
        jax.block_until_ready(out_arrs)
        return {
            nm: _np.asarray(out_arrs[i]).reshape(N_CORES, *out_avals[i].shape)
            for i, nm in enumerate(out_names)
        }

    def prepare(per_core_inputs):
        """Returns a zero-arg closure: one timed device call (wall seconds)."""
        import time as _time
        import jax.numpy as jnp
        from jax.sharding import NamedSharding

        shd = NamedSharding(mesh, PartitionSpec("core"))
        dev_in = [jax.device_put(a, shd) for a in _concat_in(per_core_inputs)]
        zero_shapes = [(N_CORES * z.shape[0], *z.shape[1:]) for z in zero_outs]
        zeros_fn = jax.jit(
            lambda: tuple(jnp.zeros(s, z.dtype)
                          for s, z in zip(zero_shapes, zero_outs)),
            out_shardings=tuple(shd for _ in zero_outs))

        def call():
            zs = jax.block_until_ready(zeros_fn())
            t0 = _time.perf_counter()
            out_arrs = sharded(*dev_in, *zs)
            jax.block_until_ready(out_arrs)
            return _time.perf_counter() - t0

        return call

    def timeit(per_core_inputs, iters=10, warmup=3):
        """Device-resident timing: returns list of per-call wall seconds."""
        call = prepare(per_core_inputs)
        times = [call() for _ in range(warmup + iters)]
        return times[warmup:]

    run.prepare = prepare
    run.timeit = timeit
    return run


def _get_runner(reps=1, dynamic=False):
    key = (reps, dynamic)
    if key not in _RUNNERS:
        _RUNNERS[key] = _make_runner(reps, dynamic)
    return _RUNNERS[key]


def _pack_core(planes):
    """[P,512,512] f32 -> partition-major 2*X fp16 [128, P, 2048]."""
    x2 = (2.0 * planes).reshape(PLANES_PER_CORE, 4, 128, N)
    return np.ascontiguousarray(
        x2.transpose(2, 0, 1, 3).reshape(128, PLANES_PER_CORE, 4 * N)
    ).astype(np.float16)


def _unpack_core(arr):
    """[128, P, 2048] fp16 -> [P, 512, 512] f32."""
    return (arr.reshape(128, PLANES_PER_CORE, 4, N)
            .transpose(1, 2, 0, 3)
            .reshape(PLANES_PER_CORE, N, N).astype(np.float32))


def _in_maps(data96):
    s1, s2 = _get_streams()
    return [
        {"data": _pack_core(data96[c * PLANES_PER_CORE:(c + 1) * PLANES_PER_CORE]),
         "s1": s1, "s2": s2}
        for c in range(N_CORES)
    ]


def _run(data96, reps=1):
    """data96: [96, 512, 512] f32. Returns [96, 512, 512] f32."""
    run = _get_runner(reps)
    outs = run(_in_maps(data96))
    return np.concatenate([_unpack_core(outs["out"][c])
                           for c in range(N_CORES)], axis=0)


def _numpy_fallback(flat):
    """Host reference path, used only if the device path raises."""
    _, G_hi = _dwt_matrices(N)
    _, S_hi = _idwt_matrices(N, M)
    E = S_hi @ G_hi
    Bm = np.eye(N) - E
    Mm = (E - np.eye(N)).T
    D = np.einsum('ik,pkl->pil', Bm, flat.astype(np.float64))
    out = 2.0 * flat + np.einsum('pil,jl->pij', D, Mm.T)
    return out.astype(np.float32)


def kernel(data):
    data = np.asarray(data, dtype=np.float32)
    flat = data.reshape(B_TOT * C_TOT, N, N)
    try:
        out = _run(flat, reps=1)
    except Exception as e:  # infrastructure failure only — keep correctness
        import sys
        print(f"WARNING: bass device path failed ({e!r}); "
              f"falling back to host numpy", file=sys.stderr)
        out = _numpy_fallback(flat)
    return out.reshape(B_TOT, C_TOT, N, N).astype(np.float32)


# revision 26
# speedup vs baseline: 1.1188x; 1.1188x over previous
"""Trainium2 Bass kernel for db4 wavelet high-frequency extraction (v2, fp16).

Math: per (b,c) plane X [512,512]:
    out = 2X + D (E-I)^T,   D = (I-E) X,   E = S_hi @ G_hi (bandwidth 7).

Device computation (per plane, all fp16 except PSUM):
  upload X2 = 2X as fp16, partition-major layout [128, plane, 4*512]
  stage 1: D2^T = X2^T B^T  (B = I-E) via banded matmuls: per 128-col strip,
           4 exact-support matmuls (548 PE cycles vs 1024 for 256-windows).
           PSUM lazy-zero semantics (start=True on the first matmul marks the
           whole bank; later matmuls overwrite pending / accumulate written
           elements) handle the band overlaps with no extra work.
  stage 2: ps = D2 (0.5(E-I))^T via the same banded scheme, then
           DVE tensor_add: out = ps + X2  (the 2X term, no identity matmul).
           Stage-1 evictions on ACT, [128,1024] 2-bank ops; output DMA
           triggered from the idle gpsimd queue (no FIFO head-of-line).
  download fp16, host converts to f32.

I/O is fp16 -> 12.58 MB per core per pass -> ~35 us HBM floor (vs 70 f32).
Per-plane emission is software-pipelined (stage2 of plane p-1 after stage1
of plane p); the timing variant unrolls UNROLL passes per For_i iteration
to amortize the loop's all-engine barrier.

Sharding: 96 (b,c) planes, 12 per core, pure data parallel on 8 cores.
"""
import numpy as np

# ---------------------------------------------------------------- constants
_DEC_LO = np.array([-0.010597401784997278, 0.032883011666982945,
                    0.030841381835986965, -0.18703481171888114,
                    -0.02798376941698385, 0.6308807679295904,
                    0.7148465705525415, 0.23037781330885523], dtype=np.float64)
_F = 8
_SIGNS = np.array([(-1.0) ** (k + 1) for k in range(_F)])
_DEC_HI = _SIGNS * _DEC_LO[::-1]
_REC_LO = _DEC_LO[::-1].copy()
_REC_HI = _DEC_HI[::-1].copy()

N = 512
M = (N + _F - 1) // 2
B_TOT, C_TOT, PLANES_PER_CORE, N_CORES = 32, 3, 12, 8
UNROLL = 24  # passes per For_i iteration in the dynamic (timing) variant

# band-block supports (BT/MT block rc nonzero cols = [SUPP[rc][0], SUPP[rc][1]))
SUPP = [(0, 134), (122, 262), (250, 390), (378, 512)]
STORE = SUPP  # stored col range per block = exact support
SBW = 144  # padded stored band width per block
# issue order: (rc, n_lo, n_hi, start).  PSUM hardware semantics: start=True
# on the FIRST matmul lazily zeroes the whole 2KB bank (per-element
# has_written bits); every later matmul overwrites still-pending elements
# and accumulates onto written ones.  So one start per bank + exact-support
# matmuls handle the band overlaps automatically, and the union of the four
# supports covers all 512 columns.
PLAN = [(0, 0, 134, True), (1, 122, 262, False),
        (2, 250, 390, False), (3, 378, 512, False)]


def _dwt_matrices(n):
    m = (n + _F - 1) // 2
    idx = np.concatenate([np.arange(_F - 2, -1, -1), np.arange(n),
                          np.arange(n - 1, n - _F, -1)])[1:]
    G_lo = np.zeros((m, n))
    G_hi = np.zeros((m, n))
    rev_lo = _DEC_LO[::-1]
    rev_hi = _DEC_HI[::-1]
    for i in range(m):
        for k in range(_F):
            t = 2 * i + k
            G_lo[i, idx[t]] += rev_lo[k]
            G_hi[i, idx[t]] += rev_hi[k]
    return G_lo, G_hi


def _idwt_matrices(n, m):
    up_len = 2 * m - 1
    S_lo = np.zeros((n, m))
    S_hi = np.zeros((n, m))
    for i in range(n):
        t = i + _F - 2
        for j_up in range(max(0, t - _F + 1), min(up_len, t + 1)):
            k = t - j_up
            if j_up % 2 == 0:
                S_lo[i, j_up // 2] += _REC_LO[k]
                S_hi[i, j_up // 2] += _REC_HI[k]
    return S_lo, S_hi


def _build_streams():
    """Returns s1, s2 [4,128,SBW] fp16: B^T and 0.5(E-I)^T band blocks."""
    _, G_hi = _dwt_matrices(N)
    _, S_hi = _idwt_matrices(N, M)
    E = S_hi @ G_hi
    BT = (np.eye(N) - E).T
    MT = 0.5 * (E - np.eye(N)).T
    s1 = np.zeros((4, 128, SBW), dtype=np.float16)
    s2 = np.zeros((4, 128, SBW), dtype=np.float16)
    for rc, (lo, hi) in enumerate(STORE):
        s1[rc, :, :hi - lo] = BT[rc * 128:(rc + 1) * 128, lo:hi]
        s2[rc, :, :hi - lo] = MT[rc * 128:(rc + 1) * 128, lo:hi]
    return s1, s2


_PLAN = PLAN


# ---------------------------------------------------------------- bass build
_NC_CACHE = {}


def _build_nc(reps=1, dynamic=False):
    import contextlib
    import concourse.bacc as bacc
    import concourse.mybir as mybir
    from concourse.tile import TileContext

    F16 = mybir.dt.float16
    F32 = mybir.dt.float32
    P = PLANES_PER_CORE

    nc = bacc.Bacc(None)
    # partition-major: data_d[p, plane, rc*512 + c] = 2*X[plane, rc*128+p, c]
    data_d = nc.declare_dram_parameter("data", [128, P, 4 * N], F16,
                                       isOutput=False)
    s1_d = nc.declare_dram_parameter("s1", [4, 128, SBW], F16, isOutput=False)
    s2_d = nc.declare_dram_parameter("s2", [4, 128, SBW], F16, isOutput=False)
    out_d = nc.declare_dram_parameter("out", [128, P, 4 * N], F16,
                                      isOutput=True)

    with TileContext(nc) as tc:
        with (
            tc.tile_pool(name="const", bufs=1) as cpool,
            tc.tile_pool(name="xin", bufs=8) as xin,
            tc.tile_pool(name="mid", bufs=4) as mid,
            tc.tile_pool(name="oout", bufs=4) as oout,
            tc.tile_pool(name="ps", bufs=2, space="PSUM") as ps,
        ):
            s1_sb = cpool.tile([128, 4, SBW], F16)
            s2_sb = cpool.tile([128, 4, SBW], F16)
            nc.sync.dma_start(out=s1_sb[:], in_=s1_d[:].rearrange("rc p w -> p rc w"))
            nc.sync.dma_start(out=s2_sb[:], in_=s2_d[:].rearrange("rc p w -> p rc w"))

            def stage1(plane):
                """Load plane, run D2^T = X2^T B^T, evict to SBUF (ACT)."""
                x_sb = xin.tile([128, 4 * N], F16, tag="x")
                # one whole-plane load (4KB contiguous per partition);
                # fewer dma_starts -> less serialized HWDGE descriptor gen
                nc.sync.dma_start(out=x_sb[:], in_=data_d[:, plane, :])
                d2t_sb = mid.tile([128, 4 * N], F16, tag="d2t")
                # 2-bank PSUM tiles; one big ACT eviction per strip-pair
                # (DVE is reserved for the stage-2 adds)
                for wq in range(2):
                    ps_t = ps.tile([128, 2 * N], F32, tag="ps_t")
                    for sub in range(2):
                        wc = 2 * wq + sub
                        for i, (rc, n0, n1, st) in enumerate(_PLAN):
                            nc.tensor.matmul(
                                ps_t[:, sub * N + n0:sub * N + n1],
                                x_sb[:, rc * N + wc * 128:rc * N + (wc + 1) * 128],
                                s1_sb[:, rc, n0 - STORE[rc][0]:n1 - STORE[rc][0]],
                                start=st, stop=(i == len(_PLAN) - 1))
                    nc.scalar.copy(
                        d2t_sb[:, wq * 2 * N:(wq + 1) * 2 * N], ps_t[:])
                return x_sb, d2t_sb

            def stage2(plane, x_sb, d2t_sb):
                """out = X2 + D2 (0.5(E-I))^T, DVE add."""
                o_sb = oout.tile([128, 4 * N], F16, tag="o")
                for iq in range(2):
                    ps_o = ps.tile([128, 2 * N], F32, tag="ps_o")
                    for sub in range(2):
                        ic = 2 * iq + sub
                        for i, (kc, n0, n1, st) in enumerate(_PLAN):
                            nc.tensor.matmul(
                                ps_o[:, sub * N + n0:sub * N + n1],
                                d2t_sb[:, kc * N + ic * 128:kc * N + (ic + 1) * 128],
                                s2_sb[:, kc, n0 - STORE[kc][0]:n1 - STORE[kc][0]],
                                start=st, stop=(i == len(_PLAN) - 1))
                    nc.vector.tensor_add(
                        o_sb[:, iq * 2 * N:(iq + 1) * 2 * N], ps_o[:],
                        x_sb[:, iq * 2 * N:(iq + 1) * 2 * N])
                # output store triggered from the (otherwise idle) gpsimd
                # queue so its wait-for-adds never head-of-line blocks the
                # ACT copies or SP loads
                nc.gpsimd.dma_start(out=out_d[:, plane, :], in_=o_sb[:])

            def one_pass():
                # software pipeline: emit stage2(p-1) after stage1(p) so the
                # strict-FIFO PE queue always has ready work (stage2's matmuls
                # wait on copies that complete during the next stage1)
                prev = None
                for plane in range(P):
                    cur = (plane, *stage1(plane))
                    if prev is not None:
                        stage2(*prev)
                    prev = cur
                stage2(*prev)

            if dynamic:
                # For_i has an all-engine barrier per iteration; unroll
                # UNROLL passes per iteration so the barrier + pipeline
                # fill/drain amortizes and passes overlap via pool rotation.
                # reps = UNROLL * n_iters must hold.
                assert reps % UNROLL == 0, (reps, UNROLL)
                with tc.For_i(0, reps // UNROLL, 1):
                    for _ in range(UNROLL):
                        one_pass()
            else:
                for _ in range(reps):
                    one_pass()

    nc.finalize()
    return nc


def _get_nc(reps=1, dynamic=False):
    key = (reps, dynamic)
    if key not in _NC_CACHE:
        _NC_CACHE[key] = _build_nc(reps, dynamic)
    return _NC_CACHE[key]


_STREAMS = None


def _get_streams():
    global _STREAMS
    if _STREAMS is None:
        _STREAMS = _build_streams()
    return _STREAMS


_RUNNERS = {}


def _make_runner(reps=1, dynamic=False):
    """Build a persistent jitted SPMD callable for the kernel program."""
    import jax
    import numpy as _np
    from jax.sharding import Mesh, PartitionSpec
    from jax.experimental.shard_map import shard_map
    import concourse.mybir as mybir
    from concourse import bass2jax

    bass2jax.install_neuronx_cc_hook()
    nc = _get_nc(reps, dynamic)

    partition_name = (nc.partition_id_tensor.name
                      if nc.partition_id_tensor else None)
    in_names, out_names, out_avals, zero_outs = [], [], [], []
    for alloc in nc.m.functions[0].allocations:
        if not isinstance(alloc, mybir.MemoryLocationSet):
            continue
        name = alloc.memorylocations[0].name
        if alloc.kind == "ExternalInput":
            if name != partition_name:
                in_names.append(name)
        elif alloc.kind == "ExternalOutput":
            out_names.append(name)
            shape = tuple(alloc.tensor_shape)
            dtype = mybir.dt.np(alloc.dtype)
            out_avals.append(jax.core.ShapedArray(shape, dtype))
            zero_outs.append(_np.zeros(shape, dtype))
    n_params = len(in_names)
    n_outs = len(out_avals)
    all_in_names = in_names + out_names
    if partition_name is not None:
        all_in_names.append(partition_name)
    donate = tuple(range(n_params, n_params + n_outs))

    def _body(*args):
        operands = list(args)
        if partition_name is not None:
            operands.append(bass2jax.partition_id_tensor())
        outs = bass2jax._bass_exec_p.bind(
            *operands,
            out_avals=tuple(out_avals),
            in_names=tuple(all_in_names),
            out_names=tuple(out_names),
            lowering_input_output_aliases=(),
            sim_require_finite=True,
            sim_require_nnan=True,
            nc=nc,
        )
        return tuple(outs)

    devices = jax.devices()[:N_CORES]
    mesh = Mesh(np.asarray(devices), ("core",))
    in_specs = (PartitionSpec("core"),) * (n_params + n_outs)
    out_specs = (PartitionSpec("core"),) * n_outs
    sharded = jax.jit(
        shard_map(_body, mesh=mesh, in_specs=in_specs, out_specs=out_specs,
                  check_rep=False),
        donate_argnums=donate, keep_unused=True)

    def _concat_in(per_core_inputs):
        return [
            _np.concatenate([_np.asarray(per_core_inputs[c][nm])
                             for c in range(N_CORES)], axis=0)
            for nm in in_names
        ]

    def run(per_core_inputs):
        """per_core_inputs: list over cores of dict name->np array."""
        concat_zeros = [
            _np.zeros((N_CORES * z.shape[0], *z.shape[1:]), z.dtype)
            for z in zero_outs
        ]
        out_arrs = sharded(*_concat_in(per_core_inputs), *concat_zeros)
        jax.block_until_ready(out_arrs)
        return {
            nm: _np.asarray(out_arrs[i]).reshape(N_CORES, *out_avals[i].shape)
            for i, nm in enumerate(out_names)
        }

    def prepare(per_core_inputs):
        """Returns a zero-arg closure: one timed device call (wall seconds)."""
        import time as _time
        import jax.numpy as jnp
        from jax.sharding import NamedSharding

        shd = NamedSharding(mesh, PartitionSpec("core"))
        dev_in = [jax.device_put(a, shd) for a in _concat_in(per_core_inputs)]
        zero_shapes = [(N_CORES * z.shape[0], *z.shape[1:]) for z in zero_outs]
        zeros_fn = jax.jit(
            lambda: tuple(jnp.zeros(s, z.dtype)
                          for s, z in zip(zero_shapes, zero_outs)),
            out_shardings=tuple(shd for _ in zero_outs))

        def call():
            zs = jax.block_until_ready(zeros_fn())
            t0 = _time.perf_counter()
            out_arrs = sharded(*dev_in, *zs)
            jax.block_until_ready(out_arrs)
            return _time.perf_counter() - t0

        return call

    def timeit(per_core_inputs, iters=10, warmup=3):
        """Device-resident timing: returns list of per-call wall seconds."""
        call = prepare(per_core_inputs)
        times = [call() for _ in range(warmup + iters)]
        return times[warmup:]

    run.prepare = prepare
    run.timeit = timeit
    return run


def _get_runner(reps=1, dynamic=False):
    key = (reps, dynamic)
    if key not in _RUNNERS:
        _RUNNERS[key] = _make_runner(reps, dynamic)
    return _RUNNERS[key]


def _pack_core(planes):
    """[P,512,512] f32 -> partition-major 2*X fp16 [128, P, 2048]."""
    x2 = (2.0 * planes).reshape(PLANES_PER_CORE, 4, 128, N)
    return np.ascontiguousarray(
        x2.transpose(2, 0, 1, 3).reshape(128, PLANES_PER_CORE, 4 * N)
    ).astype(np.float16)


def _unpack_core(arr):
    """[128, P, 2048] fp16 -> [P, 512, 512] f32."""
    return (arr.reshape(128, PLANES_PER_CORE, 4, N)
            .transpose(1, 2, 0, 3)
            .reshape(PLANES_PER_CORE, N, N).astype(np.float32))


def _in_maps(data96):
    s1, s2 = _get_streams()
    return [
        {"data": _pack_core(data96[c * PLANES_PER_CORE:(c + 1) * PLANES_PER_CORE]),
         "s1": s1, "s2": s2}
        for c in range(N_CORES)
    ]


def _run(data96, reps=1):
    """data96: [96, 512, 512] f32. Returns [96, 512, 512] f32."""
    run = _get_runner(reps)
    outs = run(_in_maps(data96))
    return np.concatenate([_unpack_core(outs["out"][c])
                           for c in range(N_CORES)], axis=0)


def _numpy_fallback(flat):
    """Host reference path, used only if the device path raises."""
    _, G_hi = _dwt_matrices(N)
    _, S_hi = _idwt_matrices(N, M)
    E = S_hi @ G_hi
    Bm = np.eye(N) - E
    Mm = (E - np.eye(N)).T
    D = np.einsum('ik,pkl->pil', Bm, flat.astype(np.float64))
    out = 2.0 * flat + np.einsum('pil,jl->pij', D, Mm.T)
    return out.astype(np.float32)


def kernel(data):
    data = np.asarray(data, dtype=np.float32)
    flat = data.reshape(B_TOT * C_TOT, N, N)
    try:
        out = _run(flat, reps=1)
    except Exception as e:  # infrastructure failure only — keep correctness
        import sys
        print(f"WARNING: bass device path failed ({e!r}); "
              f"falling back to host numpy", file=sys.stderr)
        out = _numpy_fallback(flat)
    return out.reshape(B_TOT, C_TOT, N, N).astype(np.float32)


# revision 29
# speedup vs baseline: 1.1582x; 1.0353x over previous
"""Trainium2 Bass kernel for db4 wavelet high-frequency extraction (v2, fp16).

Math: per (b,c) plane X [512,512]:
    out = 2X + D (E-I)^T,   D = (I-E) X,   E = S_hi @ G_hi (bandwidth 7).

Device computation (per plane, all fp16 except PSUM):
  upload X2 = 2X as fp16, partition-major layout [128, plane, 4*512]
  stage 1: D2^T = X2^T B^T  (B = I-E) via banded matmuls: per 128-col strip,
           4 exact-support matmuls (548 PE cycles vs 1024 for 256-windows).
           PSUM lazy-zero semantics (start=True on the first matmul marks the
           whole bank; later matmuls overwrite pending / accumulate written
           elements) handle the band overlaps with no extra work.
  stage 2: ps = D2 (0.5(E-I))^T via the same banded scheme, then
           DVE tensor_add: out = ps + X2  (the 2X term, no identity matmul).
           Stage-1 evictions on ACT, [128,1024] 2-bank ops; output DMA
           triggered from the idle gpsimd queue (no FIFO head-of-line).
  download fp16, host converts to f32.

I/O is fp16 -> 12.58 MB per core per pass -> ~35 us HBM floor (vs 70 f32).
Per-plane emission is software-pipelined (stage2 of plane p-1 after stage1
of plane p); the timing variant unrolls UNROLL passes per For_i iteration
to amortize the loop's all-engine barrier.

Sharding: 96 (b,c) planes, 12 per core, pure data parallel on 8 cores.
"""
import numpy as np

# ---------------------------------------------------------------- constants
_DEC_LO = np.array([-0.010597401784997278, 0.032883011666982945,
                    0.030841381835986965, -0.18703481171888114,
                    -0.02798376941698385, 0.6308807679295904,
                    0.7148465705525415, 0.23037781330885523], dtype=np.float64)
_F = 8
_SIGNS = np.array([(-1.0) ** (k + 1) for k in range(_F)])
_DEC_HI = _SIGNS * _DEC_LO[::-1]
_REC_LO = _DEC_LO[::-1].copy()
_REC_HI = _DEC_HI[::-1].copy()

N = 512
M = (N + _F - 1) // 2
B_TOT, C_TOT, PLANES_PER_CORE, N_CORES = 32, 3, 12, 8
UNROLL = 24  # passes per For_i iteration in the dynamic (timing) variant

# band-block supports (BT/MT block rc nonzero cols = [SUPP[rc][0], SUPP[rc][1]))
SUPP = [(0, 134), (122, 262), (250, 390), (378, 512)]
STORE = SUPP  # stored col range per block = exact support
SBW = 144  # padded stored band width per block
# issue order: (rc, n_lo, n_hi, start).  PSUM hardware semantics: start=True
# on the FIRST matmul lazily zeroes the whole 2KB bank (per-element
# has_written bits); every later matmul overwrites still-pending elements
# and accumulates onto written ones.  So one start per bank + exact-support
# matmuls handle the band overlaps automatically, and the union of the four
# supports covers all 512 columns.
PLAN = [(0, 0, 134, True), (1, 122, 262, False),
        (2, 250, 390, False), (3, 378, 512, False)]


def _dwt_matrices(n):
    m = (n + _F - 1) // 2
    idx = np.concatenate([np.arange(_F - 2, -1, -1), np.arange(n),
                          np.arange(n - 1, n - _F, -1)])[1:]
    G_lo = np.zeros((m, n))
    G_hi = np.zeros((m, n))
    rev_lo = _DEC_LO[::-1]
    rev_hi = _DEC_HI[::-1]
    for i in range(m):
        for k in range(_F):
            t = 2 * i + k
            G_lo[i, idx[t]] += rev_lo[k]
            G_hi[i, idx[t]] += rev_hi[k]
    return G_lo, G_hi


def _idwt_matrices(n, m):
    up_len = 2 * m - 1
    S_lo = np.zeros((n, m))
    S_hi = np.zeros((n, m))
    for i in range(n):
        t = i + _F - 2
        for j_up in range(max(0, t - _F + 1), min(up_len, t + 1)):
            k = t - j_up
            if j_up % 2 == 0:
                S_lo[i, j_up // 2] += _REC_LO[k]
                S_hi[i, j_up // 2] += _REC_HI[k]
    return S_lo, S_hi


def _build_streams():
    """Returns s1, s2 [4,128,SBW] fp16: B^T and 0.5(E-I)^T band blocks."""
    _, G_hi = _dwt_matrices(N)
    _, S_hi = _idwt_matrices(N, M)
    E = S_hi @ G_hi
    BT = (np.eye(N) - E).T
    MT = 0.5 * (E - np.eye(N)).T
    s1 = np.zeros((4, 128, SBW), dtype=np.float16)
    s2 = np.zeros((4, 128, SBW), dtype=np.float16)
    for rc, (lo, hi) in enumerate(STORE):
        s1[rc, :, :hi - lo] = BT[rc * 128:(rc + 1) * 128, lo:hi]
        s2[rc, :, :hi - lo] = MT[rc * 128:(rc + 1) * 128, lo:hi]
    return s1, s2


_PLAN = PLAN


# ---------------------------------------------------------------- bass build
_NC_CACHE = {}


def _build_nc(reps=1, dynamic=False):
    import contextlib
    import concourse.bacc as bacc
    import concourse.mybir as mybir
    from concourse.tile import TileContext

    F16 = mybir.dt.float16
    F32 = mybir.dt.float32
    P = PLANES_PER_CORE

    nc = bacc.Bacc(None)
    # partition-major: data_d[p, plane, rc*512 + c] = 2*X[plane, rc*128+p, c]
    data_d = nc.declare_dram_parameter("data", [128, P, 4 * N], F16,
                                       isOutput=False)
    s1_d = nc.declare_dram_parameter("s1", [4, 128, SBW], F16, isOutput=False)
    s2_d = nc.declare_dram_parameter("s2", [4, 128, SBW], F16, isOutput=False)
    out_d = nc.declare_dram_parameter("out", [128, P, 4 * N], F16,
                                      isOutput=True)

    with TileContext(nc) as tc:
        with (
            tc.tile_pool(name="const", bufs=1) as cpool,
            tc.tile_pool(name="xin", bufs=8) as xin,
            tc.tile_pool(name="mid", bufs=4) as mid,
            tc.tile_pool(name="oout", bufs=4) as oout,
            tc.tile_pool(name="ps", bufs=2, space="PSUM") as ps,
        ):
            s1_sb = cpool.tile([128, 4, SBW], F16)
            s2_sb = cpool.tile([128, 4, SBW], F16)
            nc.sync.dma_start(out=s1_sb[:], in_=s1_d[:].rearrange("rc p w -> p rc w"))
            nc.sync.dma_start(out=s2_sb[:], in_=s2_d[:].rearrange("rc p w -> p rc w"))

            def stage1(plane):
                """Load plane, run D2^T = X2^T B^T, evict to SBUF (ACT)."""
                x_sb = xin.tile([128, 4 * N], F16, tag="x")
                # one whole-plane load (4KB contiguous per partition);
                # fewer dma_starts -> less serialized HWDGE descriptor gen
                nc.sync.dma_start(out=x_sb[:], in_=data_d[:, plane, :])
                d2t_sb = mid.tile([128, 4 * N], F16, tag="d2t")
                # 2-bank PSUM tiles; one big ACT eviction per strip-pair
                # (DVE is reserved for the stage-2 adds)
                for wq in range(2):
                    ps_t = ps.tile([128, 2 * N], F32, tag="ps_t")
                    for sub in range(2):
                        wc = 2 * wq + sub
                        for i, (rc, n0, n1, st) in enumerate(_PLAN):
                            nc.tensor.matmul(
                                ps_t[:, sub * N + n0:sub * N + n1],
                                x_sb[:, rc * N + wc * 128:rc * N + (wc + 1) * 128],
                                s1_sb[:, rc, n0 - STORE[rc][0]:n1 - STORE[rc][0]],
                                start=st, stop=(i == len(_PLAN) - 1))
                    nc.scalar.copy(
                        d2t_sb[:, wq * 2 * N:(wq + 1) * 2 * N], ps_t[:])
                return x_sb, d2t_sb

            def stage2(plane, x_sb, d2t_sb):
                """out = X2 + D2 (0.5(E-I))^T, DVE add."""
                o_sb = oout.tile([128, 4 * N], F16, tag="o")
                for iq in range(2):
                    ps_o = ps.tile([128, 2 * N], F32, tag="ps_o")
                    for sub in range(2):
                        ic = 2 * iq + sub
                        for i, (kc, n0, n1, st) in enumerate(_PLAN):
                            nc.tensor.matmul(
                                ps_o[:, sub * N + n0:sub * N + n1],
                                d2t_sb[:, kc * N + ic * 128:kc * N + (ic + 1) * 128],
                                s2_sb[:, kc, n0 - STORE[kc][0]:n1 - STORE[kc][0]],
                                start=st, stop=(i == len(_PLAN) - 1))
                    nc.vector.tensor_add(
                        o_sb[:, iq * 2 * N:(iq + 1) * 2 * N], ps_o[:],
                        x_sb[:, iq * 2 * N:(iq + 1) * 2 * N])
                return plane, o_sb

            def one_pass():
                # software pipeline: emit stage2(p-1) after stage1(p) so the
                # strict-FIFO PE queue always has ready work (stage2's matmuls
                # wait on copies that complete during the next stage1).
                # Output stores alternate between the SWDGE path (gpsimd
                # queue, immediate emission) and the ACT HWDGE ring (emission
                # deferred one plane so the trigger's wait-for-adds is
                # already satisfied and never head-of-line blocks the
                # copies) — parallelizes output descriptor generation.
                pending_act = []

                def flush_act():
                    while pending_act:
                        pl, ob = pending_act.pop(0)
                        nc.scalar.dma_start(out=out_d[:, pl, :], in_=ob[:])

                def store(pl, ob):
                    if pl % 2 == 0:
                        nc.gpsimd.dma_start(out=out_d[:, pl, :], in_=ob[:])
                    else:
                        pending_act.append((pl, ob))

                prev = None
                for plane in range(P):
                    cur = (plane, *stage1(plane))
                    if prev is not None:
                        flush_act()
                        store(*stage2(*prev))
                    prev = cur
                flush_act()
                store(*stage2(*prev))
                flush_act()

            if dynamic:
                # For_i has an all-engine barrier per iteration; unroll
                # UNROLL passes per iteration so the barrier + pipeline
                # fill/drain amortizes and passes overlap via pool rotation.
                # reps = UNROLL * n_iters must hold.
                assert reps % UNROLL == 0, (reps, UNROLL)
                with tc.For_i(0, reps // UNROLL, 1):
                    for _ in range(UNROLL):
                        one_pass()
            else:
                for _ in range(reps):
                    one_pass()

    nc.finalize()
    return nc


def _get_nc(reps=1, dynamic=False):
    key = (reps, dynamic)
    if key not in _NC_CACHE:
        _NC_CACHE[key] = _build_nc(reps, dynamic)
    return _NC_CACHE[key]


_STREAMS = None


def _get_streams():
    global _STREAMS
    if _STREAMS is None:
        _STREAMS = _build_streams()
    return _STREAMS


_RUNNERS = {}


def _make_runner(reps=1, dynamic=False):
    """Build a persistent jitted SPMD callable for the kernel program."""
    import jax
    import numpy as _np
    from jax.sharding import Mesh, PartitionSpec
    from jax.experimental.shard_map import shard_map
    import concourse.mybir as mybir
    from concourse import bass2jax

    bass2jax.install_neuronx_cc_hook()
    nc = _get_nc(reps, dynamic)

    partition_name = (nc.partition_id_tensor.name
                      if nc.partition_id_tensor else None)
    in_names, out_names, out_avals, zero_outs = [], [], [], []
    for alloc in nc.m.functions[0].allocations:
        if not isinstance(alloc, mybir.MemoryLocationSet):
            continue
        name = alloc.memorylocations[0].name
        if alloc.kind == "ExternalInput":
            if name != partition_name:
                in_names.append(name)
        elif alloc.kind == "ExternalOutput":
            out_names.append(name)
            shape = tuple(alloc.tensor_shape)
            dtype = mybir.dt.np(alloc.dtype)
            out_avals.append(jax.core.ShapedArray(shape, dtype))
            zero_outs.append(_np.zeros(shape, dtype))
    n_params = len(in_names)
    n_outs = len(out_avals)
    all_in_names = in_names + out_names
    if partition_name is not None:
        all_in_names.append(partition_name)
    donate = tuple(range(n_params, n_params + n_outs))

    def _body(*args):
        operands = list(args)
        if partition_name is not None:
            operands.append(bass2jax.partition_id_tensor())
        outs = bass2jax._bass_exec_p.bind(
            *operands,
            out_avals=tuple(out_avals),
            in_names=tuple(all_in_names),
            out_names=tuple(out_names),
            lowering_input_output_aliases=(),
            sim_require_finite=True,
            sim_require_nnan=True,
            nc=nc,
        )
        return tuple(outs)

    devices = jax.devices()[:N_CORES]
    mesh = Mesh(np.asarray(devices), ("core",))
    in_specs = (PartitionSpec("core"),) * (n_params + n_outs)
    out_specs = (PartitionSpec("core"),) * n_outs
    sharded = jax.jit(
        shard_map(_body, mesh=mesh, in_specs=in_specs, out_specs=out_specs,
                  check_rep=False),
        donate_argnums=donate, keep_unused=True)

    def _concat_in(per_core_inputs):
        return [
            _np.concatenate([_np.asarray(per_core_inputs[c][nm])
                             for c in range(N_CORES)], axis=0)
            for nm in in_names
        ]

    def run(per_core_inputs):
        """per_core_inputs: list over cores of dict name->np array."""
        concat_zeros = [
            _np.zeros((N_CORES * z.shape[0], *z.shape[1:]), z.dtype)
            for z in zero_outs
        ]
        out_arrs = sharded(*_concat_in(per_core_inputs), *concat_zeros)
        jax.block_until_ready(out_arrs)
        return {
            nm: _np.asarray(out_arrs[i]).reshape(N_CORES, *out_avals[i].shape)
            for i, nm in enumerate(out_names)
        }

    def prepare(per_core_inputs):
        """Returns a zero-arg closure: one timed device call (wall seconds)."""
        import time as _time
        import jax.numpy as jnp
        from jax.sharding import NamedSharding

        shd = NamedSharding(mesh, PartitionSpec("core"))
        dev_in = [jax.device_put(a, shd) for a in _concat_in(per_core_inputs)]
        zero_shapes = [(N_CORES * z.shape[0], *z.shape[1:]) for z in zero_outs]
        zeros_fn = jax.jit(
            lambda: tuple(jnp.zeros(s, z.dtype)
                          for s, z in zip(zero_shapes, zero_outs)),
            out_shardings=tuple(shd for _ in zero_outs))

        def call():
            zs = jax.block_until_ready(zeros_fn())
            t0 = _time.perf_counter()
            out_arrs = sharded(*dev_in, *zs)
            jax.block_until_ready(out_arrs)
            return _time.perf_counter() - t0

        return call

    def timeit(per_core_inputs, iters=10, warmup=3):
        """Device-resident timing: returns list of per-call wall seconds."""
        call = prepare(per_core_inputs)
        times = [call() for _ in range(warmup + iters)]
        return times[warmup:]

    run.prepare = prepare
    run.timeit = timeit
    return run


def _get_runner(reps=1, dynamic=False):
    key = (reps, dynamic)
    if key not in _RUNNERS:
        _RUNNERS[key] = _make_runner(reps, dynamic)
    return _RUNNERS[key]


def _pack_core(planes):
    """[P,512,512] f32 -> partition-major 2*X fp16 [128, P, 2048]."""
    x2 = (2.0 * planes).reshape(PLANES_PER_CORE, 4, 128, N)
    return np.ascontiguousarray(
        x2.transpose(2, 0, 1, 3).reshape(128, PLANES_PER_CORE, 4 * N)
    ).astype(np.float16)


def _unpack_core(arr):
    """[128, P, 2048] fp16 -> [P, 512, 512] f32."""
    return (arr.reshape(128, PLANES_PER_CORE, 4, N)
            .transpose(1, 2, 0, 3)
            .reshape(PLANES_PER_CORE, N, N).astype(np.float32))


def _in_maps(data96):
    s1, s2 = _get_streams()
    return [
        {"data": _pack_core(data96[c * PLANES_PER_CORE:(c + 1) * PLANES_PER_CORE]),
         "s1": s1, "s2": s2}
        for c in range(N_CORES)
    ]


def _run(data96, reps=1):
    """data96: [96, 512, 512] f32. Returns [96, 512, 512] f32."""
    run = _get_runner(reps)
    outs = run(_in_maps(data96))
    return np.concatenate([_unpack_core(outs["out"][c])
                           for c in range(N_CORES)], axis=0)


def _numpy_fallback(flat):
    """Host reference path, used only if the device path raises."""
    _, G_hi = _dwt_matrices(N)
    _, S_hi = _idwt_matrices(N, M)
    E = S_hi @ G_hi
    Bm = np.eye(N) - E
    Mm = (E - np.eye(N)).T
    D = np.einsum('ik,pkl->pil', Bm, flat.astype(np.float64))
    out = 2.0 * flat + np.einsum('pil,jl->pij', D, Mm.T)
    return out.astype(np.float32)


def kernel(data):
    data = np.asarray(data, dtype=np.float32)
    flat = data.reshape(B_TOT * C_TOT, N, N)
    try:
        out = _run(flat, reps=1)
    except Exception as e:  # infrastructure failure only — keep correctness
        import sys
        print(f"WARNING: bass device path failed ({e!r}); "
              f"falling back to host numpy", file=sys.stderr)
        out = _numpy_fallback(flat)
    return out.reshape(B_TOT, C_TOT, N, N).astype(np.float32)


# revision 30
# speedup vs baseline: 1.1643x; 1.0053x over previous
"""Trainium2 Bass kernel for db4 wavelet high-frequency extraction (v2, fp16).

Math: per (b,c) plane X [512,512]:
    out = 2X + D (E-I)^T,   D = (I-E) X,   E = S_hi @ G_hi (bandwidth 7).

Device computation (per plane, all fp16 except PSUM):
  upload X2 = 2X as fp16, partition-major layout [128, plane, 4*512]
  stage 1: D2^T = X2^T B^T  (B = I-E) via banded matmuls: per 128-col strip,
           4 exact-support matmuls (548 PE cycles vs 1024 for 256-windows).
           PSUM lazy-zero semantics (start=True on the first matmul marks the
           whole bank; later matmuls overwrite pending / accumulate written
           elements) handle the band overlaps with no extra work.
  stage 2: ps = D2 (0.5(E-I))^T via the same banded scheme, then
           DVE tensor_add: out = ps + X2  (the 2X term, no identity matmul).
           Stage-1 evictions on ACT, [128,1024] 2-bank ops; output DMA
           triggered from the idle gpsimd queue (no FIFO head-of-line).
  download fp16, host converts to f32.

I/O is fp16 -> 12.58 MB per core per pass -> ~35 us HBM floor (vs 70 f32).
Per-plane emission is software-pipelined (stage2 of plane p-1 after stage1
of plane p); the timing variant unrolls UNROLL passes per For_i iteration
to amortize the loop's all-engine barrier.

Sharding: 96 (b,c) planes, 12 per core, pure data parallel on 8 cores.
"""
import numpy as np

# ---------------------------------------------------------------- constants
_DEC_LO = np.array([-0.010597401784997278, 0.032883011666982945,
                    0.030841381835986965, -0.18703481171888114,
                    -0.02798376941698385, 0.6308807679295904,
                    0.7148465705525415, 0.23037781330885523], dtype=np.float64)
_F = 8
_SIGNS = np.array([(-1.0) ** (k + 1) for k in range(_F)])
_DEC_HI = _SIGNS * _DEC_LO[::-1]
_REC_LO = _DEC_LO[::-1].copy()
_REC_HI = _DEC_HI[::-1].copy()

N = 512
M = (N + _F - 1) // 2
B_TOT, C_TOT, PLANES_PER_CORE, N_CORES = 32, 3, 12, 8
UNROLL = 24  # passes per For_i iteration in the dynamic (timing) variant

# band-block supports (BT/MT block rc nonzero cols = [SUPP[rc][0], SUPP[rc][1]))
SUPP = [(0, 134), (122, 262), (250, 390), (378, 512)]
STORE = SUPP  # stored col range per block = exact support
SBW = 144  # padded stored band width per block
# issue order: (rc, n_lo, n_hi, start).  PSUM hardware semantics: start=True
# on the FIRST matmul lazily zeroes the whole 2KB bank (per-element
# has_written bits); every later matmul overwrites still-pending elements
# and accumulates onto written ones.  So one start per bank + exact-support
# matmuls handle the band overlaps automatically, and the union of the four
# supports covers all 512 columns.
PLAN = [(0, 0, 134, True), (1, 122, 262, False),
        (2, 250, 390, False), (3, 378, 512, False)]


def _dwt_matrices(n):
    m = (n + _F - 1) // 2
    idx = np.concatenate([np.arange(_F - 2, -1, -1), np.arange(n),
                          np.arange(n - 1, n - _F, -1)])[1:]
    G_lo = np.zeros((m, n))
    G_hi = np.zeros((m, n))
    rev_lo = _DEC_LO[::-1]
    rev_hi = _DEC_HI[::-1]
    for i in range(m):
        for k in range(_F):
            t = 2 * i + k
            G_lo[i, idx[t]] += rev_lo[k]
            G_hi[i, idx[t]] += rev_hi[k]
    return G_lo, G_hi


def _idwt_matrices(n, m):
    up_len = 2 * m - 1
    S_lo = np.zeros((n, m))
    S_hi = np.zeros((n, m))
    for i in range(n):
        t = i + _F - 2
        for j_up in range(max(0, t - _F + 1), min(up_len, t + 1)):
            k = t - j_up
            if j_up % 2 == 0:
                S_lo[i, j_up // 2] += _REC_LO[k]
                S_hi[i, j_up // 2] += _REC_HI[k]
    return S_lo, S_hi


def _build_streams():
    """Returns s1, s2 [4,128,SBW] fp16: B^T and 0.5(E-I)^T band blocks."""
    _, G_hi = _dwt_matrices(N)
    _, S_hi = _idwt_matrices(N, M)
    E = S_hi @ G_hi
    BT = (np.eye(N) - E).T
    MT = 0.5 * (E - np.eye(N)).T
    s1 = np.zeros((4, 128, SBW), dtype=np.float16)
    s2 = np.zeros((4, 128, SBW), dtype=np.float16)
    for rc, (lo, hi) in enumerate(STORE):
        s1[rc, :, :hi - lo] = BT[rc * 128:(rc + 1) * 128, lo:hi]
        s2[rc, :, :hi - lo] = MT[rc * 128:(rc + 1) * 128, lo:hi]
    return s1, s2


_PLAN = PLAN


# ---------------------------------------------------------------- bass build
_NC_CACHE = {}


def _build_nc(reps=1, dynamic=False):
    import contextlib
    import concourse.bacc as bacc
    import concourse.mybir as mybir
    from concourse.tile import TileContext

    F16 = mybir.dt.float16
    F32 = mybir.dt.float32
    P = PLANES_PER_CORE

    nc = bacc.Bacc(None)
    # partition-major: data_d[p, plane, rc*512 + c] = 2*X[plane, rc*128+p, c]
    data_d = nc.declare_dram_parameter("data", [128, P, 4 * N], F16,
                                       isOutput=False)
    s1_d = nc.declare_dram_parameter("s1", [4, 128, SBW], F16, isOutput=False)
    s2_d = nc.declare_dram_parameter("s2", [4, 128, SBW], F16, isOutput=False)
    out_d = nc.declare_dram_parameter("out", [128, P, 4 * N], F16,
                                      isOutput=True)

    with TileContext(nc) as tc:
        with (
            tc.tile_pool(name="const", bufs=1) as cpool,
            tc.tile_pool(name="xin", bufs=8) as xin,
            tc.tile_pool(name="mid", bufs=4) as mid,
            tc.tile_pool(name="oout", bufs=4) as oout,
            tc.tile_pool(name="ps", bufs=2, space="PSUM") as ps,
        ):
            s1_sb = cpool.tile([128, 4, SBW], F16)
            s2_sb = cpool.tile([128, 4, SBW], F16)
            nc.sync.dma_start(out=s1_sb[:], in_=s1_d[:].rearrange("rc p w -> p rc w"))
            nc.sync.dma_start(out=s2_sb[:], in_=s2_d[:].rearrange("rc p w -> p rc w"))

            def stage1(plane):
                """Load plane, run D2^T = X2^T B^T, evict to SBUF (ACT)."""
                x_sb = xin.tile([128, 4 * N], F16, tag="x")
                # one whole-plane load (4KB contiguous per partition);
                # fewer dma_starts -> less serialized HWDGE descriptor gen
                nc.sync.dma_start(out=x_sb[:], in_=data_d[:, plane, :])
                d2t_sb = mid.tile([128, 4 * N], F16, tag="d2t")
                # 2-bank PSUM tiles; one big ACT eviction per strip-pair
                # (DVE is reserved for the stage-2 adds)
                for wq in range(2):
                    ps_t = ps.tile([128, 2 * N], F32, tag="ps_t")
                    for sub in range(2):
                        wc = 2 * wq + sub
                        for i, (rc, n0, n1, st) in enumerate(_PLAN):
                            nc.tensor.matmul(
                                ps_t[:, sub * N + n0:sub * N + n1],
                                x_sb[:, rc * N + wc * 128:rc * N + (wc + 1) * 128],
                                s1_sb[:, rc, n0 - STORE[rc][0]:n1 - STORE[rc][0]],
                                start=st, stop=(i == len(_PLAN) - 1))
                    nc.scalar.copy(
                        d2t_sb[:, wq * 2 * N:(wq + 1) * 2 * N], ps_t[:])
                return x_sb, d2t_sb

            def stage2(plane, x_sb, d2t_sb):
                """out = X2 + D2 (0.5(E-I))^T, DVE add."""
                o_sb = oout.tile([128, 4 * N], F16, tag="o")
                for iq in range(2):
                    ps_o = ps.tile([128, 2 * N], F32, tag="ps_o")
                    for sub in range(2):
                        ic = 2 * iq + sub
                        for i, (kc, n0, n1, st) in enumerate(_PLAN):
                            nc.tensor.matmul(
                                ps_o[:, sub * N + n0:sub * N + n1],
                                d2t_sb[:, kc * N + ic * 128:kc * N + (ic + 1) * 128],
                                s2_sb[:, kc, n0 - STORE[kc][0]:n1 - STORE[kc][0]],
                                start=st, stop=(i == len(_PLAN) - 1))
                    nc.vector.tensor_add(
                        o_sb[:, iq * 2 * N:(iq + 1) * 2 * N], ps_o[:],
                        x_sb[:, iq * 2 * N:(iq + 1) * 2 * N])
                return plane, o_sb

            def one_pass():
                # software pipeline: emit stage2(p-1) after stage1(p) so the
                # strict-FIFO PE queue always has ready work (stage2's matmuls
                # wait on copies that complete during the next stage1).
                # Output stores alternate between the SWDGE path (gpsimd
                # queue, immediate emission) and the ACT HWDGE ring (emission
                # deferred one plane so the trigger's wait-for-adds is
                # already satisfied and never head-of-line blocks the
                # copies) — parallelizes output descriptor generation.
                pending = []

                def flush_act():
                    while pending:
                        pl, ob, q = pending.pop(0)
                        q.dma_start(out=out_d[:, pl, :], in_=ob[:])

                def store(pl, ob):
                    r = pl % 3
                    if r == 0:
                        nc.gpsimd.dma_start(out=out_d[:, pl, :], in_=ob[:])
                    else:
                        pending.append((pl, ob, nc.scalar if r == 1 else nc.sync))

                prev = None
                for plane in range(P):
                    cur = (plane, *stage1(plane))
                    if prev is not None:
                        flush_act()
                        store(*stage2(*prev))
                    prev = cur
                flush_act()
                store(*stage2(*prev))
                flush_act()

            if dynamic:
                # For_i has an all-engine barrier per iteration; unroll
                # UNROLL passes per iteration so the barrier + pipeline
                # fill/drain amortizes and passes overlap via pool rotation.
                # reps = UNROLL * n_iters must hold.
                assert reps % UNROLL == 0, (reps, UNROLL)
                with tc.For_i(0, reps // UNROLL, 1):
                    for _ in range(UNROLL):
                        one_pass()
            else:
                for _ in range(reps):
                    one_pass()

    nc.finalize()
    return nc


def _get_nc(reps=1, dynamic=False):
    key = (reps, dynamic)
    if key not in _NC_CACHE:
        _NC_CACHE[key] = _build_nc(reps, dynamic)
    return _NC_CACHE[key]


_STREAMS = None


def _get_streams():
    global _STREAMS
    if _STREAMS is None:
        _STREAMS = _build_streams()
    return _STREAMS


_RUNNERS = {}


def _make_runner(reps=1, dynamic=False):
    """Build a persistent jitted SPMD callable for the kernel program."""
    import jax
    import numpy as _np
    from jax.sharding import Mesh, PartitionSpec
    from jax.experimental.shard_map import shard_map
    import concourse.mybir as mybir
    from concourse import bass2jax

    bass2jax.install_neuronx_cc_hook()
    nc = _get_nc(reps, dynamic)

    partition_name = (nc.partition_id_tensor.name
                      if nc.partition_id_tensor else None)
    in_names, out_names, out_avals, zero_outs = [], [], [], []
    for alloc in nc.m.functions[0].allocations:
        if not isinstance(alloc, mybir.MemoryLocationSet):
            continue
        name = alloc.memorylocations[0].name
        if alloc.kind == "ExternalInput":
            if name != partition_name:
                in_names.append(name)
        elif alloc.kind == "ExternalOutput":
            out_names.append(name)
            shape = tuple(alloc.tensor_shape)
            dtype = mybir.dt.np(alloc.dtype)
            out_avals.append(jax.core.ShapedArray(shape, dtype))
            zero_outs.append(_np.zeros(shape, dtype))
    n_params = len(in_names)
    n_outs = len(out_avals)
    all_in_names = in_names + out_names
    if partition_name is not None:
        all_in_names.append(partition_name)
    donate = tuple(range(n_params, n_params + n_outs))

    def _body(*args):
        operands = list(args)
        if partition_name is not None:
            operands.append(bass2jax.partition_id_tensor())
        outs = bass2jax._bass_exec_p.bind(
            *operands,
            out_avals=tuple(out_avals),
            in_names=tuple(all_in_names),
            out_names=tuple(out_names),
            lowering_input_output_aliases=(),
            sim_require_finite=True,
            sim_require_nnan=True,
            nc=nc,
        )
        return tuple(outs)

    devices = jax.devices()[:N_CORES]
    mesh = Mesh(np.asarray(devices), ("core",))
    in_specs = (PartitionSpec("core"),) * (n_params + n_outs)
    out_specs = (PartitionSpec("core"),) * n_outs
    sharded = jax.jit(
        shard_map(_body, mesh=mesh, in_specs=in_specs, out_specs=out_specs,
                  check_rep=False),
        donate_argnums=donate, keep_unused=True)

    def _concat_in(per_core_inputs):
        return [
            _np.concatenate([_np.asarray(per_core_inputs[c][nm])
                             for c in range(N_CORES)], axis=0)
            for nm in in_names
        ]

    def run(per_core_inputs):
        """per_core_inputs: list over cores of dict name->np array."""
        concat_zeros = [
            _np.zeros((N_CORES * z.shape[0], *z.shape[1:]), z.dtype)
            for z in zero_outs
        ]
        out_arrs = sharded(*_concat_in(per_core_inputs), *concat_zeros)
        jax.block_until_ready(out_arrs)
        return {
            nm: _np.asarray(out_arrs[i]).reshape(N_CORES, *out_avals[i].shape)
            for i, nm in enumerate(out_names)
        }

    def prepare(per_core_inputs):
        """Returns a zero-arg closure: one timed device call (wall seconds)."""
        import time as _time
        import jax.numpy as jnp
        from jax.sharding import NamedSharding

        shd = NamedSharding(mesh, PartitionSpec("core"))
        dev_in = [jax.device_put(a, shd) for a in _concat_in(per_core_inputs)]
        zero_shapes = [(N_CORES * z.shape[0], *z.shape[1:]) for z in zero_outs]
        zeros_fn = jax.jit(
            lambda: tuple(jnp.zeros(s, z.dtype)
                          for s, z in zip(zero_shapes, zero_outs)),
            out_shardings=tuple(shd for _ in zero_outs))

        def call():
            zs = jax.block_until_ready(zeros_fn())
            t0 = _time.perf_counter()
            out_arrs = sharded(*dev_in, *zs)
            jax.block_until_ready(out_arrs)
            return _time.perf_counter() - t0

        return call

    def timeit(per_core_inputs, iters=10, warmup=3):
        """Device-resident timing: returns list of per-call wall seconds."""
        call = prepare(per_core_inputs)
        times = [call() for _ in range(warmup + iters)]
        return times[warmup:]

    run.prepare = prepare
    run.timeit = timeit
    return run


def _get_runner(reps=1, dynamic=False):
    key = (reps, dynamic)
    if key not in _RUNNERS:
        _RUNNERS[key] = _make_runner(reps, dynamic)
    return _RUNNERS[key]


def _pack_core(planes):
    """[P,512,512] f32 -> partition-major 2*X fp16 [128, P, 2048]."""
    x2 = (2.0 * planes).reshape(PLANES_PER_CORE, 4, 128, N)
    return np.ascontiguousarray(
        x2.transpose(2, 0, 1, 3).reshape(128, PLANES_PER_CORE, 4 * N)
    ).astype(np.float16)


def _unpack_core(arr):
    """[128, P, 2048] fp16 -> [P, 512, 512] f32."""
    return (arr.reshape(128, PLANES_PER_CORE, 4, N)
            .transpose(1, 2, 0, 3)
            .reshape(PLANES_PER_CORE, N, N).astype(np.float32))


def _in_maps(data96):
    s1, s2 = _get_streams()
    return [
        {"data": _pack_core(data96[c * PLANES_PER_CORE:(c + 1) * PLANES_PER_CORE]),
         "s1": s1, "s2": s2}
        for c in range(N_CORES)
    ]


def _run(data96, reps=1):
    """data96: [96, 512, 512] f32. Returns [96, 512, 512] f32."""
    run = _get_runner(reps)
    outs = run(_in_maps(data96))
    return np.concatenate([_unpack_core(outs["out"][c])
                           for c in range(N_CORES)], axis=0)


def _numpy_fallback(flat):
    """Host reference path, used only if the device path raises."""
    _, G_hi = _dwt_matrices(N)
    _, S_hi = _idwt_matrices(N, M)
    E = S_hi @ G_hi
    Bm = np.eye(N) - E
    Mm = (E - np.eye(N)).T
    D = np.einsum('ik,pkl->pil', Bm, flat.astype(np.float64))
    out = 2.0 * flat + np.einsum('pil,jl->pij', D, Mm.T)
    return out.astype(np.float32)


def kernel(data):
    data = np.asarray(data, dtype=np.float32)
    flat = data.reshape(B_TOT * C_TOT, N, N)
    try:
        out = _run(flat, reps=1)
    except Exception as e:  # infrastructure failure only — keep correctness
        import sys
        print(f"WARNING: bass device path failed ({e!r}); "
              f"falling back to host numpy", file=sys.stderr)
        out = _numpy_fallback(flat)
    return out.reshape(B_TOT, C_TOT, N, N).astype(np.float32)
